# revision 1
# baseline (speedup 1.0000x reference)
"""GCNNet (SimpleConv sum-aggr + global_mean_pool + 2-layer MLP) on 8 trn2 cores.

Math: out[g] = MLP(relu(sums[g] / max(counts[g],1)))
  sums[g,:]  = sum_e w_e * x[src_e,:] * [batch[dst_e]==g]
  counts[g]  = #{i : batch[i]==g}

Sharding: by src-node range (6250 nodes/core).  The host reformats the edge
list into dense per-window coefficient blocks (placement only, no arithmetic):
for each 128-node window w, C_w[p, g] = w_e for the first edge with
(src==node(w,p), batch[dst]==g); duplicate (src,g) edges are placed in extra
"dup windows" whose lhsT rows replicate the needed x rows, one edge per row.
On device each window is one PE matmul accT[96,512] += x_w^T @ C_w with f32
PSUM accumulation; counts are built on-device from batch values via
is_equal(iota) one-hots; an AllReduce over the 8 cores combines the partial
[97,512] (96 feature rows + 1 count row); every core then runs the tiny MLP.
"""

import numpy as np

N_NODES = 50000
N_EDGES = 800000
D_FEAT = 96
D_HID = 10
N_GRAPHS = 512
CORES = 8
NPC = N_NODES // CORES          # 6250 nodes per core
MAIN_W = (NPC + 127) // 128     # 49 windows of 128 nodes
P = 128
CHUNK = 8                       # windows per streamed DMA chunk

# low-precision dtype for the heavy matmul operands ("float16" | "float32")
LO_DT = "float16"

_nc_cache = {}


def _build_nc(tot_w, lo_name):
    import concourse.mybir as mybir
    import concourse.tile as tile
    from concourse import bacc

    f32 = mybir.dt.float32
    lo = getattr(mybir.dt, lo_name)
    G = N_GRAPHS
    D = D_FEAT

    nc = bacc.Bacc(
        "TRN2",
        target_bir_lowering=False,
        debug=False,
        num_devices=CORES,
    )

    xw_d = nc.dram_tensor("xw", [P, tot_w * D], lo, kind="ExternalInput")
    cw_d = nc.dram_tensor("cw", [P, tot_w * G], lo, kind="ExternalInput")
    bw_d = nc.dram_tensor("bw", [P, MAIN_W], lo, kind="ExternalInput")
    iota_d = nc.dram_tensor("iota", [P, G], lo, kind="ExternalInput")
    w1_d = nc.dram_tensor("w1", [D, D_HID], f32, kind="ExternalInput")
    b1_d = nc.dram_tensor("b1", [D_HID, 1], f32, kind="ExternalInput")
    w2_d = nc.dram_tensor("w2", [D_HID, 1], f32, kind="ExternalInput")
    b2_d = nc.dram_tensor("b2", [1, 1], f32, kind="ExternalInput")
    out_d = nc.dram_tensor("out", [1, G], f32, kind="ExternalOutput")

    n_chunks = (tot_w + CHUNK - 1) // CHUNK

    with tile.TileContext(nc) as tc:
        with (
            tc.tile_pool(name="const", bufs=1) as cp,
            tc.tile_pool(name="xs", bufs=3) as xs_pool,
            tc.tile_pool(name="cs", bufs=3) as cs_pool,
            tc.tile_pool(name="oh", bufs=4) as oh_pool,
            tc.tile_pool(name="psum", bufs=1, space="PSUM") as pp,
            tc.tile_pool(name="dram", bufs=1, space="DRAM") as dp,
        ):
            bw_t = cp.tile([P, MAIN_W], lo, tag="bw")
            nc.sync.dma_start(out=bw_t[:], in_=bw_d[:, :])
            iota_t = cp.tile([P, G], lo, tag="iota")
            nc.sync.dma_start(out=iota_t[:], in_=iota_d[:, :])
            w1_t = cp.tile([D, D_HID], f32, tag="w1")
            nc.sync.dma_start(out=w1_t[:], in_=w1_d[:, :])
            b1_t = cp.tile([D_HID, 1], f32, tag="b1")
            nc.sync.dma_start(out=b1_t[:], in_=b1_d[:, :])
            w2_t = cp.tile([D_HID, 1], f32, tag="w2")
            nc.sync.dma_start(out=w2_t[:], in_=w2_d[:, :])
            b2_t = cp.tile([1, 1], f32, tag="b2")
            nc.sync.dma_start(out=b2_t[:], in_=b2_d[:, :])

            ones_t = cp.tile([P, 1], lo, tag="ones")
            nc.vector.memset(ones_t[:], 1.0)
            ones10_t = cp.tile([1, D_HID], f32, tag="ones10")
            nc.vector.memset(ones10_t[:], 1.0)

            acc_ps = pp.tile([D, G], f32, tag="acc")
            cnt_ps = pp.tile([1, G], f32, tag="cnt")

            for c in range(n_chunks):
                w0 = c * CHUNK
                w1_ = min(tot_w, w0 + CHUNK)
                nw = w1_ - w0
                xt = xs_pool.tile([P, CHUNK * D], lo, tag="xs")
                nc.sync.dma_start(out=xt[:, : nw * D], in_=xw_d[:, w0 * D : w1_ * D])
                ct = cs_pool.tile([P, CHUNK * G], lo, tag="cs")
                nc.sync.dma_start(out=ct[:, : nw * G], in_=cw_d[:, w0 * G : w1_ * G])
                for lw in range(nw):
                    w = w0 + lw
                    nc.tensor.matmul(
                        acc_ps[:, :],
                        lhsT=xt[:, lw * D : (lw + 1) * D],
                        rhs=ct[:, lw * G : (lw + 1) * G],
                        start=(w == 0),
                        stop=(w == tot_w - 1),
                    )
                    if w < MAIN_W:
                        oh = oh_pool.tile([P, G], lo, tag="oh")
                        nc.vector.tensor_tensor(
                            oh[:],
                            iota_t[:],
                            bw_t[:, w : w + 1].to_broadcast([P, G]),
                            mybir.AluOpType.is_equal,
                        )
                        nc.tensor.matmul(
                            cnt_ps[:, :],
                            lhsT=ones_t[:],
                            rhs=oh[:],
                            start=(w == 0),
                            stop=(w == MAIN_W - 1),
                        )

            # pack [97, G]: rows 0..95 = partial sums^T, row 96 = partial counts
            red_sb = cp.tile([D + 1, G], f32, tag="red")
            nc.vector.tensor_copy(out=red_sb[0:D, :], in_=acc_ps[:, :])
            nc.vector.tensor_copy(out=red_sb[D : D + 1, :], in_=cnt_ps[:, :])

            cc_in = dp.tile([D + 1, G], f32)
            cc_out = dp.tile([D + 1, G], f32)
            nc.gpsimd.dma_start(out=cc_in[:], in_=red_sb[:])
            nc.gpsimd.collective_compute(
                "AllReduce",
                mybir.AluOpType.add,
                replica_groups=[list(range(CORES))],
                ins=[cc_in.opt()],
                outs=[cc_out.opt()],
            )
            allr = cp.tile([D + 1, G], f32, tag="allr")
            nc.gpsimd.dma_start(out=allr[:], in_=cc_out[:])

            # epilogue: relu commutes with the positive per-graph 1/count scale:
            # relu(sums/c) @ W1 = (1/c) * (relu(sums) @ W1)
            a_sb = cp.tile([D, G], f32, tag="a")
            nc.vector.tensor_scalar_max(a_sb[:], allr[0:D, :], 0.0)
            cmax = cp.tile([1, G], f32, tag="cmax")
            nc.vector.tensor_scalar_max(cmax[:], allr[D : D + 1, :], 1.0)
            recip = cp.tile([1, G], f32, tag="recip")
            nc.vector.reciprocal(recip[:], cmax[:])

            b_ps = pp.tile([D_HID, G], f32, tag="b")
            nc.tensor.matmul(b_ps[:, :], lhsT=w1_t[:], rhs=a_sb[:], start=True, stop=True)
            rb_ps = pp.tile([D_HID, G], f32, tag="rb")
            nc.tensor.matmul(
                rb_ps[:, :], lhsT=ones10_t[:], rhs=recip[:], start=True, stop=True
            )
            rb_sb = cp.tile([D_HID, G], f32, tag="rbs")
            nc.vector.tensor_copy(out=rb_sb[:, :], in_=rb_ps[:, :])

            z_sb = cp.tile([D_HID, G], f32, tag="z")
            nc.vector.tensor_tensor(
                z_sb[:], b_ps[:, :], rb_sb[:], mybir.AluOpType.mult
            )
            nc.vector.tensor_scalar(
                out=z_sb[:],
                in0=z_sb[:],
                scalar1=b1_t[:],
                scalar2=0.0,
                op0=mybir.AluOpType.add,
                op1=mybir.AluOpType.max,
            )

            o_ps = pp.tile([1, G], f32, tag="o")
            nc.tensor.matmul(o_ps[:, :], lhsT=w2_t[:], rhs=z_sb[:], start=True, stop=True)
            o_sb = cp.tile([1, G], f32, tag="os")
            nc.vector.tensor_scalar(
                out=o_sb[:],
                in0=o_ps[:, :],
                scalar1=b2_t[:],
                scalar2=None,
                op0=mybir.AluOpType.add,
            )
            nc.sync.dma_start(out=out_d[:, :], in_=o_sb[:])

    nc.compile()
    return nc


def prepare_inputs(x, edge_index, edge_attr, batch, W1, b1, W2, b2, lo_name=None):
    """Host-side reformatting (placement only): per-core window tensors."""
    lo = np.float16 if (lo_name or LO_DT) == "float16" else np.float32
    G = N_GRAPHS
    D = D_FEAT

    x = np.asarray(x, np.float32)
    src = np.asarray(edge_index[0], np.int64)
    dst = np.asarray(edge_index[1], np.int64)
    w = np.asarray(edge_attr, np.float32)
    batch = np.asarray(batch, np.int64)
    g = batch[dst]

    core = src // NPC
    per_core = []
    max_dups = 0
    for k in range(CORES):
        m = core == k
        s_loc = (src[m] - k * NPC).astype(np.int64)
        gk = g[m]
        wk = w[m]
        key = s_loc * G + gk
        _, first_idx = np.unique(key, return_index=True)
        is_first = np.zeros(len(key), dtype=bool)
        is_first[first_idx] = True
        per_core.append((k, s_loc, gk, wk, is_first))
        max_dups = max(max_dups, int((~is_first).sum()))

    dup_w = max(1, -(-max_dups // P))
    tot_w = MAIN_W + dup_w

    iota = np.broadcast_to(
        np.arange(G, dtype=np.float32), (P, G)
    ).astype(lo)

    in_maps = []
    for k, s_loc, gk, wk, is_first in per_core:
        # lhsT windows: x rows laid out [P, tot_w*D]
        xw = np.zeros((P, tot_w * D), dtype=lo)
        xk = np.zeros((MAIN_W * P, D), dtype=np.float32)
        xk[:NPC] = x[k * NPC : (k + 1) * NPC]
        xw[:, : MAIN_W * D] = (
            xk.reshape(MAIN_W, P, D).transpose(1, 0, 2).reshape(P, MAIN_W * D)
        )

        cw = np.zeros((P, tot_w * G), dtype=lo)
        # main placement: first edge per (local node, graph)
        sf = s_loc[is_first]
        gf = gk[is_first]
        wf = wk[is_first]
        cw[sf % P, (sf // P) * G + gf] = wf.astype(lo)
        # duplicates: one edge per slot in dup windows
        sd = s_loc[~is_first]
        gd = gk[~is_first]
        wd = wk[~is_first]
        i = np.arange(len(sd))
        cw[i % P, (MAIN_W + i // P) * G + gd] = wd.astype(lo)
        dup_x = np.zeros((dup_w * P, D), dtype=np.float32)
        dup_x[: len(sd)] = x[k * NPC + sd]
        xw[:, MAIN_W * D :] = (
            dup_x.reshape(dup_w, P, D).transpose(1, 0, 2).reshape(P, dup_w * D)
        )

        bwk = np.full(MAIN_W * P, -1.0, dtype=np.float32)
        bwk[:NPC] = batch[k * NPC : (k + 1) * NPC].astype(np.float32)
        bw = bwk.reshape(MAIN_W, P).T.copy().astype(lo)

        in_maps.append(
            {
                "xw": xw,
                "cw": cw,
                "bw": np.ascontiguousarray(bw),
                "iota": np.ascontiguousarray(iota),
                "w1": np.asarray(W1, np.float32).reshape(D, D_HID),
                "b1": np.asarray(b1, np.float32).reshape(D_HID, 1),
                "w2": np.asarray(W2, np.float32).reshape(D_HID, 1),
                "b2": np.asarray(b2, np.float32).reshape(1, 1),
            }
        )
    return in_maps, tot_w


def get_nc(tot_w, lo_name=None):
    lo_name = lo_name or LO_DT
    key = (tot_w, lo_name)
    if key not in _nc_cache:
        _nc_cache[key] = _build_nc(tot_w, lo_name)
    return _nc_cache[key]


def kernel(**inputs):
    from concourse import bass_utils

    in_maps, tot_w = prepare_inputs(**inputs)
    nc = get_nc(tot_w)
    res = bass_utils.run_bass_kernel_spmd(nc, in_maps, core_ids=list(range(CORES)))
    out = res.results[0]["out"]
    return np.asarray(out, np.float32).reshape(N_GRAPHS, 1)


if __name__ == "__main__":
    rng = np.random.default_rng(0)
    x = rng.standard_normal((N_NODES, D_FEAT), dtype=np.float32)
    edge_index = rng.integers(0, N_NODES, (2, N_EDGES), dtype=np.int64)
    edge_attr = rng.random(N_EDGES, dtype=np.float32)
    batch = np.sort(rng.integers(0, N_GRAPHS, N_NODES, dtype=np.int64))
    W1 = rng.standard_normal((D_FEAT, D_HID), dtype=np.float32) / np.sqrt(D_FEAT)
    b1 = rng.standard_normal(D_HID, dtype=np.float32) * 0.01
    W2 = rng.standard_normal((D_HID, 1), dtype=np.float32) / np.sqrt(D_HID)
    b2 = rng.standard_normal(1, dtype=np.float32) * 0.01
    out = kernel(
        x=x, edge_index=edge_index, edge_attr=edge_attr, batch=batch,
        W1=W1, b1=b1, W2=W2, b2=b2,
    )
    print(out.shape, out[:5, 0])


# revision 2
# speedup vs baseline: 1.0287x; 1.0287x over previous
"""GCNNet (SimpleConv sum-aggr + global_mean_pool + 2-layer MLP) on 8 trn2 cores.

Math: out[g] = MLP(relu(sums[g] / max(counts[g],1)))
  sums[g,:]  = sum_e w_e * x[src_e,:] * [batch[dst_e]==g]
  counts[g]  = #{i : batch[i]==g}

Sharding: by src-node range (6250 nodes/core).  The host reformats the edge
list into dense per-window coefficient blocks (placement only, no arithmetic):
for each 128-node window w, C_w[p, g] = w_e for the first edge with
(src==node(w,p), batch[dst]==g); duplicate (src,g) edges are placed in extra
"dup windows" whose lhsT rows replicate the needed x rows, one edge per row.
On device each window is one PE matmul accT[96,512] += x_w^T @ C_w with f32
PSUM accumulation.  Node counts per graph are 0/1 "multiplicity layer"
matrices (host placement; batch is sorted so few layers suffice), reduced on
device by ones^T @ layer matmuls.  An AllReduce over the 8 cores combines the
partial [97,512] (96 feature rows + 1 count row); every core then runs the
tiny MLP epilogue on-device.
"""

import numpy as np

N_NODES = 50000
N_EDGES = 800000
D_FEAT = 96
D_HID = 10
N_GRAPHS = 512
CORES = 8
NPC = N_NODES // CORES          # 6250 nodes per core
MAIN_W = (NPC + 127) // 128     # 49 windows of 128 nodes
P = 128
CHUNK = 8                       # windows per streamed DMA chunk

# low-precision dtype for the heavy matmul operands ("float16" | "float32")
LO_DT = "float16"

_nc_cache = {}


def _build_nc(tot_w, n_cnt_layers, lo_name):
    import concourse.mybir as mybir
    import concourse.tile as tile
    from concourse import bacc

    f32 = mybir.dt.float32
    lo = getattr(mybir.dt, lo_name)
    G = N_GRAPHS
    D = D_FEAT
    L = n_cnt_layers

    nc = bacc.Bacc(
        "TRN2",
        target_bir_lowering=False,
        debug=False,
        num_devices=CORES,
    )

    xw_d = nc.dram_tensor("xw", [P, tot_w * D], lo, kind="ExternalInput")
    cw_d = nc.dram_tensor("cw", [P, tot_w * G], lo, kind="ExternalInput")
    cm_d = nc.dram_tensor("cm", [P, L * G], lo, kind="ExternalInput")
    w1_d = nc.dram_tensor("w1", [D, D_HID], f32, kind="ExternalInput")
    b1_d = nc.dram_tensor("b1", [D_HID, 1], f32, kind="ExternalInput")
    w2_d = nc.dram_tensor("w2", [D_HID, 1], f32, kind="ExternalInput")
    b2_d = nc.dram_tensor("b2", [1, 1], f32, kind="ExternalInput")
    out_d = nc.dram_tensor("out", [1, G], f32, kind="ExternalOutput")

    n_chunks = (tot_w + CHUNK - 1) // CHUNK

    with tile.TileContext(nc) as tc:
        with (
            tc.tile_pool(name="const", bufs=1) as cp,
            tc.tile_pool(name="xs", bufs=3) as xs_pool,
            tc.tile_pool(name="cs", bufs=3) as cs_pool,
            tc.tile_pool(name="psum", bufs=1, space="PSUM") as pp,
            tc.tile_pool(name="dram", bufs=1, space="DRAM") as dp,
        ):
            acc_ps = pp.tile([D, G], f32, tag="acc")
            cnt_ps = pp.tile([1, G], f32, tag="cnt")

            ones_t = cp.tile([P, 1], lo, tag="ones")
            nc.vector.memset(ones_t[:], 1.0)
            ones10_t = cp.tile([1, D_HID], f32, tag="ones10")
            nc.vector.memset(ones10_t[:], 1.0)

            # chunked window stream: DMA chunk c, then its matmuls
            cm_t = None
            for c in range(n_chunks):
                w0 = c * CHUNK
                w1_ = min(tot_w, w0 + CHUNK)
                nw = w1_ - w0
                xt = xs_pool.tile([P, CHUNK * D], lo, tag="xs")
                nc.sync.dma_start(out=xt[:, : nw * D], in_=xw_d[:, w0 * D : w1_ * D])
                ct = cs_pool.tile([P, CHUNK * G], lo, tag="cs")
                nc.sync.dma_start(out=ct[:, : nw * G], in_=cw_d[:, w0 * G : w1_ * G])
                if c == 0:
                    # small consts after the first chunk is queued
                    cm_t = cp.tile([P, L * G], lo, tag="cm")
                    nc.sync.dma_start(out=cm_t[:], in_=cm_d[:, :])
                    w1_t = cp.tile([D, D_HID], f32, tag="w1")
                    nc.sync.dma_start(out=w1_t[:], in_=w1_d[:, :])
                    b1_t = cp.tile([D_HID, 1], f32, tag="b1")
                    nc.sync.dma_start(out=b1_t[:], in_=b1_d[:, :])
                    w2_t = cp.tile([D_HID, 1], f32, tag="w2")
                    nc.sync.dma_start(out=w2_t[:], in_=w2_d[:, :])
                    b2_t = cp.tile([1, 1], f32, tag="b2")
                    nc.sync.dma_start(out=b2_t[:], in_=b2_d[:, :])
                for lw in range(nw):
                    w = w0 + lw
                    nc.tensor.matmul(
                        acc_ps[:, :],
                        lhsT=xt[:, lw * D : (lw + 1) * D],
                        rhs=ct[:, lw * G : (lw + 1) * G],
                        start=(w == 0),
                        stop=(w == tot_w - 1),
                    )

            # node counts: L layer matmuls
            for l in range(L):
                nc.tensor.matmul(
                    cnt_ps[:, :],
                    lhsT=ones_t[:],
                    rhs=cm_t[:, l * G : (l + 1) * G],
                    start=(l == 0),
                    stop=(l == L - 1),
                )

            # pack [97, G]: rows 0..95 = partial sums^T, row 96 = partial counts
            red_sb = cp.tile([D + 1, G], f32, tag="red")
            nc.vector.tensor_copy(out=red_sb[0:D, :], in_=acc_ps[:, :])
            nc.vector.tensor_copy(out=red_sb[D : D + 1, :], in_=cnt_ps[:, :])

            cc_in = dp.tile([D + 1, G], f32)
            cc_out = dp.tile([D + 1, G], f32)
            nc.sync.dma_start(out=cc_in[:], in_=red_sb[:])
            nc.gpsimd.collective_compute(
                "AllReduce",
                mybir.AluOpType.add,
                replica_groups=[list(range(CORES))],
                ins=[cc_in.opt()],
                outs=[cc_out.opt()],
            )
            allr = cp.tile([D + 1, G], f32, tag="allr")
            nc.sync.dma_start(out=allr[:], in_=cc_out[:])

            # epilogue: relu commutes with the positive per-graph 1/count scale:
            # relu(sums/c) @ W1 = (1/c) * (relu(sums) @ W1)
            a_sb = cp.tile([D, G], f32, tag="a")
            nc.vector.tensor_scalar_max(a_sb[:], allr[0:D, :], 0.0)
            cmax = cp.tile([1, G], f32, tag="cmax")
            nc.vector.tensor_scalar_max(cmax[:], allr[D : D + 1, :], 1.0)
            recip = cp.tile([1, G], f32, tag="recip")
            nc.vector.reciprocal(recip[:], cmax[:])

            b_ps = pp.tile([D_HID, G], f32, tag="b")
            nc.tensor.matmul(b_ps[:, :], lhsT=w1_t[:], rhs=a_sb[:], start=True, stop=True)
            rb_ps = pp.tile([D_HID, G], f32, tag="rb")
            nc.tensor.matmul(
                rb_ps[:, :], lhsT=ones10_t[:], rhs=recip[:], start=True, stop=True
            )
            rb_sb = cp.tile([D_HID, G], f32, tag="rbs")
            nc.vector.tensor_copy(out=rb_sb[:, :], in_=rb_ps[:, :])

            z_sb = cp.tile([D_HID, G], f32, tag="z")
            nc.vector.tensor_tensor(
                z_sb[:], b_ps[:, :], rb_sb[:], mybir.AluOpType.mult
            )
            nc.vector.tensor_scalar(
                out=z_sb[:],
                in0=z_sb[:],
                scalar1=b1_t[:],
                scalar2=0.0,
                op0=mybir.AluOpType.add,
                op1=mybir.AluOpType.max,
            )

            o_ps = pp.tile([1, G], f32, tag="o")
            nc.tensor.matmul(o_ps[:, :], lhsT=w2_t[:], rhs=z_sb[:], start=True, stop=True)
            o_sb = cp.tile([1, G], f32, tag="os")
            nc.vector.tensor_scalar(
                out=o_sb[:],
                in0=o_ps[:, :],
                scalar1=b2_t[:],
                scalar2=None,
                op0=mybir.AluOpType.add,
            )
            nc.sync.dma_start(out=out_d[:, :], in_=o_sb[:])

    nc.compile()
    return nc


def _occurrence_ranks(key):
    """rank of each element within its equal-key group (0-based), stable."""
    order = np.argsort(key, kind="stable")
    sk = key[order]
    n = len(sk)
    if n == 0:
        return np.zeros(0, np.int64)
    starts = np.r_[0, np.flatnonzero(np.diff(sk)) + 1]
    lens = np.diff(np.r_[starts, n])
    ranks_sorted = np.arange(n) - np.repeat(starts, lens)
    ranks = np.empty(n, np.int64)
    ranks[order] = ranks_sorted
    return ranks


def prepare_inputs(x, edge_index, edge_attr, batch, W1, b1, W2, b2, lo_name=None):
    """Host-side reformatting (placement only): per-core window tensors."""
    lo = np.float16 if (lo_name or LO_DT) == "float16" else np.float32
    G = N_GRAPHS
    D = D_FEAT

    x = np.asarray(x, np.float32)
    src = np.asarray(edge_index[0], np.int64)
    dst = np.asarray(edge_index[1], np.int64)
    w = np.asarray(edge_attr, np.float32)
    batch = np.asarray(batch, np.int64)
    g = batch[dst]

    core = src // NPC
    per_core = []
    max_dups = 0
    max_layers = 0
    for k in range(CORES):
        m = core == k
        s_loc = (src[m] - k * NPC).astype(np.int64)
        gk = g[m]
        wk = w[m]
        key = s_loc * G + gk
        _, first_idx = np.unique(key, return_index=True)
        is_first = np.zeros(len(key), dtype=bool)
        is_first[first_idx] = True
        per_core.append((k, s_loc, gk, wk, is_first))
        max_dups = max(max_dups, int((~is_first).sum()))
        # count-layer depth for this core's nodes
        bk = batch[k * NPC : (k + 1) * NPC]
        pk = np.arange(NPC) % P
        ranks = _occurrence_ranks(pk * G + bk)
        max_layers = max(max_layers, int(ranks.max()) + 1)

    dup_w = max(1, -(-max_dups // P))
    tot_w = MAIN_W + dup_w
    n_layers = max_layers
    assert n_layers <= 6, n_layers

    in_maps = []
    for k, s_loc, gk, wk, is_first in per_core:
        # lhsT windows: x rows laid out [P, tot_w*D]
        xw = np.zeros((P, tot_w * D), dtype=lo)
        xk = np.zeros((MAIN_W * P, D), dtype=np.float32)
        xk[:NPC] = x[k * NPC : (k + 1) * NPC]
        xw[:, : MAIN_W * D] = (
            xk.reshape(MAIN_W, P, D).transpose(1, 0, 2).reshape(P, MAIN_W * D)
        )

        cw = np.zeros((P, tot_w * G), dtype=lo)
        # main placement: first edge per (local node, graph)
        sf = s_loc[is_first]
        gf = gk[is_first]
        wf = wk[is_first]
        cw[sf % P, (sf // P) * G + gf] = wf.astype(lo)
        # duplicates: one edge per slot in dup windows
        sd = s_loc[~is_first]
        gd = gk[~is_first]
        wd = wk[~is_first]
        i = np.arange(len(sd))
        cw[i % P, (MAIN_W + i // P) * G + gd] = wd.astype(lo)
        dup_x = np.zeros((dup_w * P, D), dtype=np.float32)
        dup_x[: len(sd)] = x[k * NPC + sd]
        xw[:, MAIN_W * D :] = (
            dup_x.reshape(dup_w, P, D).transpose(1, 0, 2).reshape(P, dup_w * D)
        )

        # count layers: 0/1 placement, r-th occurrence of (p, batch) -> layer r
        bk = batch[k * NPC : (k + 1) * NPC]
        pk = np.arange(NPC) % P
        ranks = _occurrence_ranks(pk * G + bk)
        cm = np.zeros((P, n_layers * G), dtype=lo)
        cm[pk, ranks * G + bk] = 1.0

        in_maps.append(
            {
                "xw": xw,
                "cw": cw,
                "cm": cm,
                "w1": np.asarray(W1, np.float32).reshape(D, D_HID),
                "b1": np.asarray(b1, np.float32).reshape(D_HID, 1),
                "w2": np.asarray(W2, np.float32).reshape(D_HID, 1),
                "b2": np.asarray(b2, np.float32).reshape(1, 1),
            }
        )
    return in_maps, tot_w, n_layers


def get_nc(tot_w, n_layers, lo_name=None):
    lo_name = lo_name or LO_DT
    key = (tot_w, n_layers, lo_name)
    if key not in _nc_cache:
        _nc_cache[key] = _build_nc(tot_w, n_layers, lo_name)
    return _nc_cache[key]


def kernel(**inputs):
    from concourse import bass_utils

    in_maps, tot_w, n_layers = prepare_inputs(**inputs)
    nc = get_nc(tot_w, n_layers)
    res = bass_utils.run_bass_kernel_spmd(nc, in_maps, core_ids=list(range(CORES)))
    out = res.results[0]["out"]
    return np.asarray(out, np.float32).reshape(N_GRAPHS, 1)


# revision 3
# speedup vs baseline: 1.9915x; 1.9360x over previous
"""GCNNet (SimpleConv sum-aggr + global_mean_pool + 2-layer MLP) on 8 trn2 cores.

Math: out[g] = MLP(relu(sums[g] / max(counts[g],1)))
  sums[g,:]  = sum_e w_e * x[src_e,:] * [batch[dst_e]==g]
  counts[g]  = #{i : batch[i]==g}

Sharding: by graph range (64 graphs per core) -> fully independent cores, no
collective.  The host reformats each core's edge list into dense window
blocks (placement only, no arithmetic): rows are (src, layer) pairs holding a
copy of x[src]; for each row-window w a dense C_w[p, 0:64] holds w_e at the
edge's local graph column (duplicate (src,g) edges get their own row layer so
every edge keeps its own cell).  On device each window is one PE matmul
accT[96,64] += x_w^T @ C_w with f32 PSUM accumulation.  Node counts per graph
come from 0/1 "multiplicity layer" matrices (host placement; batch is sorted
so 2-3 layers suffice) reduced by ones^T @ layer matmuls.  Each core then
runs the tiny MLP epilogue for its 64 graphs; the host concatenates.
"""

import numpy as np

N_NODES = 50000
N_EDGES = 800000
D_FEAT = 96
D_HID = 10
N_GRAPHS = 512
CORES = 8
GPC = N_GRAPHS // CORES         # 64 graphs per core
P = 128

# low-precision dtype for the heavy matmul operands ("float16" | "float32")
LO_DT = "float16"

_nc_cache = {}


def _chunks(tot_w):
    """window chunks: small first chunk for an early PE start."""
    out = []
    w = 0
    first = True
    while w < tot_w:
        n = min(8 if first else 32, tot_w - w)
        out.append((w, n))
        w += n
        first = False
    return out


def _build_nc(tot_w, n_cnt_layers, lo_name):
    import concourse.mybir as mybir
    import concourse.tile as tile
    from concourse import bacc

    f32 = mybir.dt.float32
    lo = getattr(mybir.dt, lo_name)
    G = GPC
    D = D_FEAT
    L = n_cnt_layers

    nc = bacc.Bacc(
        "TRN2",
        target_bir_lowering=False,
        debug=False,
        num_devices=CORES,
    )

    xw_d = nc.dram_tensor("xw", [P, tot_w * D], lo, kind="ExternalInput")
    cw_d = nc.dram_tensor("cw", [P, tot_w * G], lo, kind="ExternalInput")
    cm_d = nc.dram_tensor("cm", [P, L * G], lo, kind="ExternalInput")
    w1_d = nc.dram_tensor("w1", [D, D_HID], f32, kind="ExternalInput")
    b1_d = nc.dram_tensor("b1", [D_HID, 1], f32, kind="ExternalInput")
    w2_d = nc.dram_tensor("w2", [D_HID, 1], f32, kind="ExternalInput")
    b2_d = nc.dram_tensor("b2", [1, 1], f32, kind="ExternalInput")
    out_d = nc.dram_tensor("out", [1, G], f32, kind="ExternalOutput")

    with tile.TileContext(nc) as tc:
        with (
            tc.tile_pool(name="const", bufs=1) as cp,
            tc.tile_pool(name="xs", bufs=4) as xs_pool,
            tc.tile_pool(name="cs", bufs=4) as cs_pool,
            tc.tile_pool(name="psum", bufs=1, space="PSUM") as pp,
        ):
            acc_ps = pp.tile([D, G], f32, tag="acc")
            cnt_ps = pp.tile([1, G], f32, tag="cnt")

            ones_t = cp.tile([P, 1], lo, tag="ones")
            nc.vector.memset(ones_t[:], 1.0)
            ones10_t = cp.tile([1, D_HID], f32, tag="ones10")
            nc.vector.memset(ones10_t[:], 1.0)

            chunks = _chunks(tot_w)
            cm_t = None
            for c, (w0, nw) in enumerate(chunks):
                w1_ = w0 + nw
                xt = xs_pool.tile([P, 32 * D], lo, tag="xs")
                nc.sync.dma_start(out=xt[:, : nw * D], in_=xw_d[:, w0 * D : w1_ * D])
                ct = cs_pool.tile([P, 32 * G], lo, tag="cs")
                nc.sync.dma_start(out=ct[:, : nw * G], in_=cw_d[:, w0 * G : w1_ * G])
                if c == 0:
                    # small consts after the first chunk is queued
                    cm_t = cp.tile([P, L * G], lo, tag="cm")
                    nc.sync.dma_start(out=cm_t[:], in_=cm_d[:, :])
                    w1_t = cp.tile([D, D_HID], f32, tag="w1")
                    nc.sync.dma_start(out=w1_t[:], in_=w1_d[:, :])
                    b1_t = cp.tile([D_HID, 1], f32, tag="b1")
                    nc.sync.dma_start(out=b1_t[:], in_=b1_d[:, :])
                    w2_t = cp.tile([D_HID, 1], f32, tag="w2")
                    nc.sync.dma_start(out=w2_t[:], in_=w2_d[:, :])
                    b2_t = cp.tile([1, 1], f32, tag="b2")
                    nc.sync.dma_start(out=b2_t[:], in_=b2_d[:, :])
                for lw in range(nw):
                    w = w0 + lw
                    nc.tensor.matmul(
                        acc_ps[:, :],
                        lhsT=xt[:, lw * D : (lw + 1) * D],
                        rhs=ct[:, lw * G : (lw + 1) * G],
                        start=(w == 0),
                        stop=(w == tot_w - 1),
                    )

            # node counts: L layer matmuls
            for l in range(L):
                nc.tensor.matmul(
                    cnt_ps[:, :],
                    lhsT=ones_t[:],
                    rhs=cm_t[:, l * G : (l + 1) * G],
                    start=(l == 0),
                    stop=(l == L - 1),
                )

            # epilogue: relu commutes with the positive per-graph 1/count scale:
            # relu(sums/c) @ W1 = (1/c) * (relu(sums) @ W1)
            a_sb = cp.tile([D, G], f32, tag="a")
            nc.vector.tensor_scalar_max(a_sb[:], acc_ps[:, :], 0.0)
            cmax = cp.tile([1, G], f32, tag="cmax")
            nc.vector.tensor_scalar_max(cmax[:], cnt_ps[:, :], 1.0)
            recip = cp.tile([1, G], f32, tag="recip")
            nc.vector.reciprocal(recip[:], cmax[:])

            b_ps = pp.tile([D_HID, G], f32, tag="b")
            nc.tensor.matmul(b_ps[:, :], lhsT=w1_t[:], rhs=a_sb[:], start=True, stop=True)
            rb_ps = pp.tile([D_HID, G], f32, tag="rb")
            nc.tensor.matmul(
                rb_ps[:, :], lhsT=ones10_t[:], rhs=recip[:], start=True, stop=True
            )
            rb_sb = cp.tile([D_HID, G], f32, tag="rbs")
            nc.vector.tensor_copy(out=rb_sb[:, :], in_=rb_ps[:, :])

            z_sb = cp.tile([D_HID, G], f32, tag="z")
            nc.vector.tensor_tensor(
                z_sb[:], b_ps[:, :], rb_sb[:], mybir.AluOpType.mult
            )
            nc.vector.tensor_scalar(
                out=z_sb[:],
                in0=z_sb[:],
                scalar1=b1_t[:],
                scalar2=0.0,
                op0=mybir.AluOpType.add,
                op1=mybir.AluOpType.max,
            )

            o_ps = pp.tile([1, G], f32, tag="o")
            nc.tensor.matmul(o_ps[:, :], lhsT=w2_t[:], rhs=z_sb[:], start=True, stop=True)
            o_sb = cp.tile([1, G], f32, tag="os")
            nc.vector.tensor_scalar(
                out=o_sb[:],
                in0=o_ps[:, :],
                scalar1=b2_t[:],
                scalar2=None,
                op0=mybir.AluOpType.add,
            )
            nc.sync.dma_start(out=out_d[:, :], in_=o_sb[:])

    nc.compile()
    return nc


def _occurrence_ranks(key):
    """rank of each element within its equal-key group (0-based), stable."""
    order = np.argsort(key, kind="stable")
    sk = key[order]
    n = len(sk)
    if n == 0:
        return np.zeros(0, np.int64)
    starts = np.r_[0, np.flatnonzero(np.diff(sk)) + 1]
    lens = np.diff(np.r_[starts, n])
    ranks_sorted = np.arange(n) - np.repeat(starts, lens)
    ranks = np.empty(n, np.int64)
    ranks[order] = ranks_sorted
    return ranks


def prepare_inputs(x, edge_index, edge_attr, batch, W1, b1, W2, b2, lo_name=None):
    """Host-side reformatting (placement only): per-core window tensors."""
    lo = np.float16 if (lo_name or LO_DT) == "float16" else np.float32
    G = GPC
    D = D_FEAT

    x = np.asarray(x, np.float32)
    src = np.asarray(edge_index[0], np.int64)
    dst = np.asarray(edge_index[1], np.int64)
    w = np.asarray(edge_attr, np.float32)
    batch = np.asarray(batch, np.int64)
    g = batch[dst]

    core = g // G
    MAXR = 16  # max copies of one (src, graph) pair handled per row layer key
    per_core = []
    max_rows = 0
    max_layers = 0
    # node range per core: batch is sorted
    node_bounds = np.searchsorted(batch, np.arange(CORES + 1) * G)
    for k in range(CORES):
        m = core == k
        sk_ = src[m]
        gk = (g[m] - k * G).astype(np.int64)
        wk = w[m]
        # rank of each edge within its (src, g) duplicate group
        r = _occurrence_ranks(sk_ * (G * MAXR) + gk)
        assert r.max(initial=0) < MAXR
        # row = (src, r): shared by all of src's rank-r edges (distinct g)
        row_key = sk_ * MAXR + r
        uniq, row_of_edge = np.unique(row_key, return_inverse=True)
        max_rows = max(max_rows, len(uniq))
        per_core.append((k, uniq, row_of_edge, gk, wk))

        n0, n1 = node_bounds[k], node_bounds[k + 1]
        bk = batch[n0:n1] - k * G
        pk = np.arange(n1 - n0) % P
        ranks = _occurrence_ranks(pk * G + bk)
        max_layers = max(max_layers, int(ranks.max(initial=-1)) + 1)

    tot_w = max(1, -(-max_rows // P))
    n_layers = max(1, max_layers)
    assert n_layers <= 6, n_layers

    in_maps = []
    for k, uniq, row_of_edge, gk, wk in per_core:
        nrows = len(uniq)
        row_src = uniq // MAXR  # the x row each window-row holds

        xr = np.zeros((tot_w * P, D), dtype=np.float32)
        xr[:nrows] = x[row_src]
        xw = np.ascontiguousarray(
            xr.reshape(tot_w, P, D).transpose(1, 0, 2).reshape(P, tot_w * D)
        ).astype(lo)

        cw = np.zeros((P, tot_w * G), dtype=lo)
        cw[row_of_edge % P, (row_of_edge // P) * G + gk] = wk.astype(lo)

        # count layers: 0/1 placement, r-th occurrence of (p, batch) -> layer r
        n0, n1 = node_bounds[k], node_bounds[k + 1]
        bk = batch[n0:n1] - k * G
        pk = np.arange(n1 - n0) % P
        ranks = _occurrence_ranks(pk * G + bk)
        cm = np.zeros((P, n_layers * G), dtype=lo)
        cm[pk, ranks * G + bk] = 1.0

        in_maps.append(
            {
                "xw": xw,
                "cw": cw,
                "cm": cm,
                "w1": np.asarray(W1, np.float32).reshape(D_FEAT, D_HID),
                "b1": np.asarray(b1, np.float32).reshape(D_HID, 1),
                "w2": np.asarray(W2, np.float32).reshape(D_HID, 1),
                "b2": np.asarray(b2, np.float32).reshape(1, 1),
            }
        )
    return in_maps, tot_w, n_layers


def get_nc(tot_w, n_layers, lo_name=None):
    lo_name = lo_name or LO_DT
    key = (tot_w, n_layers, lo_name)
    if key not in _nc_cache:
        _nc_cache[key] = _build_nc(tot_w, n_layers, lo_name)
    return _nc_cache[key]


def kernel(**inputs):
    from concourse import bass_utils

    in_maps, tot_w, n_layers = prepare_inputs(**inputs)
    nc = get_nc(tot_w, n_layers)
    res = bass_utils.run_bass_kernel_spmd(nc, in_maps, core_ids=list(range(CORES)))
    out = np.concatenate(
        [np.asarray(res.results[k]["out"], np.float32).reshape(GPC) for k in range(CORES)]
    )
    return out.reshape(N_GRAPHS, 1)


# revision 6
# speedup vs baseline: 2.0669x; 1.0378x over previous
"""GCNNet (SimpleConv sum-aggr + global_mean_pool + 2-layer MLP) on 8 trn2 cores.

Math: out[g] = MLP(relu(sums[g] / max(counts[g],1)))
  sums[g,:]  = sum_e w_e * x[src_e,:] * [batch[dst_e]==g]
  counts[g]  = #{i : batch[i]==g}

Sharding: by graph range (64 graphs per core) -> fully independent cores, no
collective.  The host reformats each core's edge list into dense window
blocks (placement only, no arithmetic): rows are (src, layer) pairs holding a
copy of x[src]; for each row-window w a dense C_w[p, 0:64] holds w_e at the
edge's local graph column (duplicate (src,g) edges get their own row layer so
every edge keeps its own cell).  On device each window is one PE matmul
accT[96,64] += x_w^T @ C_w with f32 PSUM accumulation.  Node counts per graph
come from 0/1 "multiplicity layer" matrices (host placement; batch is sorted
so 2-3 layers suffice) reduced by ones^T @ layer matmuls.  Each core then
runs the tiny MLP epilogue for its 64 graphs; the host concatenates.
"""

import numpy as np

N_NODES = 50000
N_EDGES = 800000
D_FEAT = 96
D_HID = 10
N_GRAPHS = 512
CORES = 8
GPC = N_GRAPHS // CORES         # 64 graphs per core
P = 128

# low-precision dtype for the heavy matmul operands ("float16" | "float32")
LO_DT = "float16"

_nc_cache = {}


def _chunks(tot_w):
    """window chunks: small first chunk for an early PE start."""
    out = []
    w = 0
    first = True
    while w < tot_w:
        n = min(16 if first else 32, tot_w - w)
        out.append((w, n))
        w += n
        first = False
    return out


def _build_nc(tot_w, n_cnt_layers, lo_name):
    import concourse.mybir as mybir
    import concourse.tile as tile
    from concourse import bacc

    f32 = mybir.dt.float32
    lo = getattr(mybir.dt, lo_name)
    G = GPC
    D = D_FEAT
    L = n_cnt_layers

    nc = bacc.Bacc(
        "TRN2",
        target_bir_lowering=False,
        debug=False,
        num_devices=CORES,
    )

    xw_d = nc.dram_tensor("xw", [P, tot_w * D], lo, kind="ExternalInput")
    cw_d = nc.dram_tensor("cw", [P, tot_w * G], lo, kind="ExternalInput")
    cm_d = nc.dram_tensor("cm", [P, L * G], lo, kind="ExternalInput")
    w1_d = nc.dram_tensor("w1", [D, D_HID], f32, kind="ExternalInput")
    b1_d = nc.dram_tensor("b1", [D_HID, 1], f32, kind="ExternalInput")
    w2_d = nc.dram_tensor("w2", [D_HID, 1], f32, kind="ExternalInput")
    b2_d = nc.dram_tensor("b2", [1, 1], f32, kind="ExternalInput")
    out_d = nc.dram_tensor("out", [1, G], f32, kind="ExternalOutput")

    with tile.TileContext(nc) as tc:
        with (
            tc.tile_pool(name="const", bufs=1) as cp,
            tc.tile_pool(name="xs", bufs=6) as xs_pool,
            tc.tile_pool(name="cs", bufs=6) as cs_pool,
            tc.tile_pool(name="psum", bufs=1, space="PSUM") as pp,
        ):
            acc_ps = pp.tile([D, G], f32, tag="acc")
            cnt_ps = pp.tile([1, G], f32, tag="cnt")

            ones_t = cp.tile([P, 1], lo, tag="ones")
            nc.vector.memset(ones_t[:], 1.0)
            ones10_t = cp.tile([1, D_HID], f32, tag="ones10")
            nc.vector.memset(ones10_t[:], 1.0)

            chunks = _chunks(tot_w)
            cm_t = None
            for c, (w0, nw) in enumerate(chunks):
                w1_ = w0 + nw
                xt = xs_pool.tile([P, 32 * D], lo, tag="xs")
                nc.sync.dma_start(out=xt[:, : nw * D], in_=xw_d[:, w0 * D : w1_ * D])
                ct = cs_pool.tile([P, 32 * G], lo, tag="cs")
                nc.sync.dma_start(out=ct[:, : nw * G], in_=cw_d[:, w0 * G : w1_ * G])
                if c == 2:
                    # small consts once the pipeline is primed (only needed
                    # for the count matmuls and the epilogue)
                    cm_t = cp.tile([P, L * G], lo, tag="cm")
                    nc.sync.dma_start(out=cm_t[:], in_=cm_d[:, :])
                    w1_t = cp.tile([D, D_HID], f32, tag="w1")
                    nc.sync.dma_start(out=w1_t[:], in_=w1_d[:, :])
                    b1_t = cp.tile([D_HID, 1], f32, tag="b1")
                    nc.sync.dma_start(out=b1_t[:], in_=b1_d[:, :])
                    w2_t = cp.tile([D_HID, 1], f32, tag="w2")
                    nc.sync.dma_start(out=w2_t[:], in_=w2_d[:, :])
                    b2_t = cp.tile([1, 1], f32, tag="b2")
                    nc.sync.dma_start(out=b2_t[:], in_=b2_d[:, :])
                for lw in range(nw):
                    w = w0 + lw
                    nc.tensor.matmul(
                        acc_ps[:, :],
                        lhsT=xt[:, lw * D : (lw + 1) * D],
                        rhs=ct[:, lw * G : (lw + 1) * G],
                        start=(w == 0),
                        stop=(w == tot_w - 1),
                    )

            # node counts: L layer matmuls
            for l in range(L):
                nc.tensor.matmul(
                    cnt_ps[:, :],
                    lhsT=ones_t[:],
                    rhs=cm_t[:, l * G : (l + 1) * G],
                    start=(l == 0),
                    stop=(l == L - 1),
                )

            # epilogue: relu commutes with the positive per-graph 1/count scale:
            # relu(sums/c) @ W1 = (1/c) * (relu(sums) @ W1)
            a_sb = cp.tile([D, G], f32, tag="a")
            nc.vector.tensor_scalar_max(a_sb[:], acc_ps[:, :], 0.0)
            cmax = cp.tile([1, G], f32, tag="cmax")
            nc.vector.tensor_scalar_max(cmax[:], cnt_ps[:, :], 1.0)
            recip = cp.tile([1, G], f32, tag="recip")
            nc.vector.reciprocal(recip[:], cmax[:])

            b_ps = pp.tile([D_HID, G], f32, tag="b")
            nc.tensor.matmul(b_ps[:, :], lhsT=w1_t[:], rhs=a_sb[:], start=True, stop=True)
            rb_ps = pp.tile([D_HID, G], f32, tag="rb")
            nc.tensor.matmul(
                rb_ps[:, :], lhsT=ones10_t[:], rhs=recip[:], start=True, stop=True
            )
            rb_sb = cp.tile([D_HID, G], f32, tag="rbs")
            nc.vector.tensor_copy(out=rb_sb[:, :], in_=rb_ps[:, :])

            z_sb = cp.tile([D_HID, G], f32, tag="z")
            nc.vector.tensor_tensor(
                z_sb[:], b_ps[:, :], rb_sb[:], mybir.AluOpType.mult
            )
            nc.vector.tensor_scalar(
                out=z_sb[:],
                in0=z_sb[:],
                scalar1=b1_t[:],
                scalar2=0.0,
                op0=mybir.AluOpType.add,
                op1=mybir.AluOpType.max,
            )

            o_ps = pp.tile([1, G], f32, tag="o")
            nc.tensor.matmul(o_ps[:, :], lhsT=w2_t[:], rhs=z_sb[:], start=True, stop=True)
            o_sb = cp.tile([1, G], f32, tag="os")
            nc.vector.tensor_scalar(
                out=o_sb[:],
                in0=o_ps[:, :],
                scalar1=b2_t[:],
                scalar2=None,
                op0=mybir.AluOpType.add,
            )
            nc.sync.dma_start(out=out_d[:, :], in_=o_sb[:])

    nc.compile()
    return nc


def _occurrence_ranks(key):
    """rank of each element within its equal-key group (0-based), stable."""
    order = np.argsort(key, kind="stable")
    sk = key[order]
    n = len(sk)
    if n == 0:
        return np.zeros(0, np.int64)
    starts = np.r_[0, np.flatnonzero(np.diff(sk)) + 1]
    lens = np.diff(np.r_[starts, n])
    ranks_sorted = np.arange(n) - np.repeat(starts, lens)
    ranks = np.empty(n, np.int64)
    ranks[order] = ranks_sorted
    return ranks


def prepare_inputs(x, edge_index, edge_attr, batch, W1, b1, W2, b2, lo_name=None):
    """Host-side reformatting (placement only): per-core window tensors."""
    lo = np.float16 if (lo_name or LO_DT) == "float16" else np.float32
    G = GPC
    D = D_FEAT

    x = np.asarray(x, np.float32)
    src = np.asarray(edge_index[0], np.int64)
    dst = np.asarray(edge_index[1], np.int64)
    w = np.asarray(edge_attr, np.float32)
    batch = np.asarray(batch, np.int64)
    g = batch[dst]

    core = g // G
    MAXR = 16  # max copies of one (src, graph) pair handled per row layer key
    per_core = []
    max_rows = 0
    max_layers = 0
    # node range per core: batch is sorted
    node_bounds = np.searchsorted(batch, np.arange(CORES + 1) * G)
    for k in range(CORES):
        m = core == k
        sk_ = src[m]
        gk = (g[m] - k * G).astype(np.int64)
        wk = w[m]
        # rank of each edge within its (src, g) duplicate group
        r = _occurrence_ranks(sk_ * (G * MAXR) + gk)
        assert r.max(initial=0) < MAXR
        # row = (src, r): shared by all of src's rank-r edges (distinct g)
        row_key = sk_ * MAXR + r
        uniq, row_of_edge = np.unique(row_key, return_inverse=True)
        max_rows = max(max_rows, len(uniq))
        per_core.append((k, uniq, row_of_edge, gk, wk))

        n0, n1 = node_bounds[k], node_bounds[k + 1]
        bk = batch[n0:n1] - k * G
        pk = np.arange(n1 - n0) % P
        ranks = _occurrence_ranks(pk * G + bk)
        max_layers = max(max_layers, int(ranks.max(initial=-1)) + 1)

    tot_w = max(1, -(-max_rows // P))
    n_layers = max(1, max_layers)
    assert n_layers <= 6, n_layers

    in_maps = []
    for k, uniq, row_of_edge, gk, wk in per_core:
        nrows = len(uniq)
        row_src = uniq // MAXR  # the x row each window-row holds

        xr = np.zeros((tot_w * P, D), dtype=np.float32)
        xr[:nrows] = x[row_src]
        xw = np.ascontiguousarray(
            xr.reshape(tot_w, P, D).transpose(1, 0, 2).reshape(P, tot_w * D)
        ).astype(lo)

        cw = np.zeros((P, tot_w * G), dtype=lo)
        cw[row_of_edge % P, (row_of_edge // P) * G + gk] = wk.astype(lo)

        # count layers: 0/1 placement, r-th occurrence of (p, batch) -> layer r
        n0, n1 = node_bounds[k], node_bounds[k + 1]
        bk = batch[n0:n1] - k * G
        pk = np.arange(n1 - n0) % P
        ranks = _occurrence_ranks(pk * G + bk)
        cm = np.zeros((P, n_layers * G), dtype=lo)
        cm[pk, ranks * G + bk] = 1.0

        in_maps.append(
            {
                "xw": xw,
                "cw": cw,
                "cm": cm,
                "w1": np.asarray(W1, np.float32).reshape(D_FEAT, D_HID),
                "b1": np.asarray(b1, np.float32).reshape(D_HID, 1),
                "w2": np.asarray(W2, np.float32).reshape(D_HID, 1),
                "b2": np.asarray(b2, np.float32).reshape(1, 1),
            }
        )
    return in_maps, tot_w, n_layers


def get_nc(tot_w, n_layers, lo_name=None):
    lo_name = lo_name or LO_DT
    key = (tot_w, n_layers, lo_name)
    if key not in _nc_cache:
        _nc_cache[key] = _build_nc(tot_w, n_layers, lo_name)
    return _nc_cache[key]


def kernel(**inputs):
    from concourse import bass_utils

    in_maps, tot_w, n_layers = prepare_inputs(**inputs)
    nc = get_nc(tot_w, n_layers)
    res = bass_utils.run_bass_kernel_spmd(nc, in_maps, core_ids=list(range(CORES)))
    out = np.concatenate(
        [np.asarray(res.results[k]["out"], np.float32).reshape(GPC) for k in range(CORES)]
    )
    return out.reshape(N_GRAPHS, 1)


# revision 12
# speedup vs baseline: 2.0678x; 1.0004x over previous
"""GCNNet (SimpleConv sum-aggr + global_mean_pool + 2-layer MLP) on 8 trn2 cores.

Math: out[g] = MLP(relu(sums[g] / max(counts[g],1)))
  sums[g,:]  = sum_e w_e * x[src_e,:] * [batch[dst_e]==g]
  counts[g]  = #{i : batch[i]==g}

Sharding: by graph range (64 graphs per core) -> fully independent cores, no
collective.  The host reformats each core's edge list into dense window
blocks (placement only, no arithmetic): rows are (src, layer) pairs holding a
copy of x[src]; for each row-window w a dense C_w[p, 0:64] holds w_e at the
edge's local graph column (duplicate (src,g) edges get their own row layer so
every edge keeps its own cell).  On device each window is one PE matmul
accT[96,64] += x_w^T @ C_w with f32 PSUM accumulation.  Node counts per graph
come from 0/1 "multiplicity layer" matrices (host placement; batch is sorted
so 2-3 layers suffice) reduced by ones^T @ layer matmuls.  Each core then
runs the tiny MLP epilogue for its 64 graphs; the host concatenates.
"""

import numpy as np

N_NODES = 50000
N_EDGES = 800000
D_FEAT = 96
D_HID = 10
N_GRAPHS = 512
CORES = 8
GPC = N_GRAPHS // CORES         # 64 graphs per core
P = 128

# low-precision dtype for the heavy matmul operands ("float16" | "float32")
LO_DT = "float16"

_nc_cache = {}


def _chunks(tot_w):
    """window chunks: ramped sizes for an early PE start."""
    sizes = [8, 16, 32, 48]
    out = []
    w = 0
    i = 0
    while w < tot_w:
        n = min(sizes[i] if i < len(sizes) else 64, tot_w - w)
        out.append((w, n))
        w += n
        i += 1
    return out


def _build_nc(tot_w, n_cnt_layers, lo_name):
    import concourse.mybir as mybir
    import concourse.tile as tile
    from concourse import bacc

    f32 = mybir.dt.float32
    lo = getattr(mybir.dt, lo_name)
    G = GPC
    D = D_FEAT
    L = n_cnt_layers

    nc = bacc.Bacc(
        "TRN2",
        target_bir_lowering=False,
        debug=False,
        num_devices=CORES,
    )

    DG = D + G
    xc_d = nc.dram_tensor("xc", [P, tot_w * DG], lo, kind="ExternalInput")
    cm_d = nc.dram_tensor("cm", [P, L * G], lo, kind="ExternalInput")
    w1_d = nc.dram_tensor("w1", [D, D_HID], f32, kind="ExternalInput")
    b1_d = nc.dram_tensor("b1", [D_HID, 1], f32, kind="ExternalInput")
    w2_d = nc.dram_tensor("w2", [D_HID, 1], f32, kind="ExternalInput")
    b2_d = nc.dram_tensor("b2", [1, 1], f32, kind="ExternalInput")
    out_d = nc.dram_tensor("out", [1, G], f32, kind="ExternalOutput")

    with tile.TileContext(nc) as tc:
        with (
            tc.tile_pool(name="const", bufs=1) as cp,
            tc.tile_pool(name="xc", bufs=4) as xc_pool,
            tc.tile_pool(name="psum", bufs=1, space="PSUM") as pp,
        ):
            acc_ps = pp.tile([D, G], f32, tag="acc")
            cnt_ps = pp.tile([1, G], f32, tag="cnt")

            ones_t = cp.tile([P, 1], lo, tag="ones")
            nc.vector.memset(ones_t[:], 1.0)
            ones10_t = cp.tile([1, D_HID], f32, tag="ones10")
            nc.vector.memset(ones10_t[:], 1.0)

            chunks = _chunks(tot_w)
            cm_t = None
            for c, (w0, nw) in enumerate(chunks):
                w1_ = w0 + nw
                xt = xc_pool.tile([P, 64 * DG], lo, tag="xc")
                nc.sync.dma_start(out=xt[:, : nw * DG], in_=xc_d[:, w0 * DG : w1_ * DG])
                if c == 2:
                    # small consts once the pipeline is primed (only needed
                    # for the count matmuls and the epilogue)
                    cm_t = cp.tile([P, L * G], lo, tag="cm")
                    nc.sync.dma_start(out=cm_t[:], in_=cm_d[:, :])
                    w1_t = cp.tile([D, D_HID], f32, tag="w1")
                    nc.sync.dma_start(out=w1_t[:], in_=w1_d[:, :])
                    b1_t = cp.tile([D_HID, 1], f32, tag="b1")
                    nc.sync.dma_start(out=b1_t[:], in_=b1_d[:, :])
                    w2_t = cp.tile([D_HID, 1], f32, tag="w2")
                    nc.sync.dma_start(out=w2_t[:], in_=w2_d[:, :])
                    b2_t = cp.tile([1, 1], f32, tag="b2")
                    nc.sync.dma_start(out=b2_t[:], in_=b2_d[:, :])
                for lw in range(nw):
                    w = w0 + lw
                    nc.tensor.matmul(
                        acc_ps[:, :],
                        lhsT=xt[:, lw * DG : lw * DG + D],
                        rhs=xt[:, lw * DG + D : (lw + 1) * DG],
                        start=(w == 0),
                        stop=(w == tot_w - 1),
                    )

            # node counts: L layer matmuls
            for l in range(L):
                nc.tensor.matmul(
                    cnt_ps[:, :],
                    lhsT=ones_t[:],
                    rhs=cm_t[:, l * G : (l + 1) * G],
                    start=(l == 0),
                    stop=(l == L - 1),
                )

            # epilogue: relu commutes with the positive per-graph 1/count scale:
            # relu(sums/c) @ W1 = (1/c) * (relu(sums) @ W1)
            a_sb = cp.tile([D, G], f32, tag="a")
            nc.vector.tensor_scalar_max(a_sb[:], acc_ps[:, :], 0.0)
            cmax = cp.tile([1, G], f32, tag="cmax")
            nc.vector.tensor_scalar_max(cmax[:], cnt_ps[:, :], 1.0)
            recip = cp.tile([1, G], f32, tag="recip")
            nc.vector.reciprocal(recip[:], cmax[:])

            b_ps = pp.tile([D_HID, G], f32, tag="b")
            nc.tensor.matmul(b_ps[:, :], lhsT=w1_t[:], rhs=a_sb[:], start=True, stop=True)
            rb_ps = pp.tile([D_HID, G], f32, tag="rb")
            nc.tensor.matmul(
                rb_ps[:, :], lhsT=ones10_t[:], rhs=recip[:], start=True, stop=True
            )
            rb_sb = cp.tile([D_HID, G], f32, tag="rbs")
            nc.vector.tensor_copy(out=rb_sb[:, :], in_=rb_ps[:, :])

            z_sb = cp.tile([D_HID, G], f32, tag="z")
            nc.vector.tensor_tensor(
                z_sb[:], b_ps[:, :], rb_sb[:], mybir.AluOpType.mult
            )
            nc.vector.tensor_scalar(
                out=z_sb[:],
                in0=z_sb[:],
                scalar1=b1_t[:],
                scalar2=0.0,
                op0=mybir.AluOpType.add,
                op1=mybir.AluOpType.max,
            )

            o_ps = pp.tile([1, G], f32, tag="o")
            nc.tensor.matmul(o_ps[:, :], lhsT=w2_t[:], rhs=z_sb[:], start=True, stop=True)
            o_sb = cp.tile([1, G], f32, tag="os")
            nc.vector.tensor_scalar(
                out=o_sb[:],
                in0=o_ps[:, :],
                scalar1=b2_t[:],
                scalar2=None,
                op0=mybir.AluOpType.add,
            )
            nc.sync.dma_start(out=out_d[:, :], in_=o_sb[:])

    nc.compile()
    return nc


def _occurrence_ranks(key):
    """rank of each element within its equal-key group (0-based), stable."""
    order = np.argsort(key, kind="stable")
    sk = key[order]
    n = len(sk)
    if n == 0:
        return np.zeros(0, np.int64)
    starts = np.r_[0, np.flatnonzero(np.diff(sk)) + 1]
    lens = np.diff(np.r_[starts, n])
    ranks_sorted = np.arange(n) - np.repeat(starts, lens)
    ranks = np.empty(n, np.int64)
    ranks[order] = ranks_sorted
    return ranks


def prepare_inputs(x, edge_index, edge_attr, batch, W1, b1, W2, b2, lo_name=None):
    """Host-side reformatting (placement only): per-core window tensors."""
    lo = np.float16 if (lo_name or LO_DT) == "float16" else np.float32
    G = GPC
    D = D_FEAT

    x = np.asarray(x, np.float32)
    src = np.asarray(edge_index[0], np.int64)
    dst = np.asarray(edge_index[1], np.int64)
    w = np.asarray(edge_attr, np.float32)
    batch = np.asarray(batch, np.int64)
    g = batch[dst]

    core = g // G
    MAXR = 16  # max copies of one (src, graph) pair handled per row layer key
    per_core = []
    max_rows = 0
    max_layers = 0
    # node range per core: batch is sorted
    node_bounds = np.searchsorted(batch, np.arange(CORES + 1) * G)
    for k in range(CORES):
        m = core == k
        sk_ = src[m]
        gk = (g[m] - k * G).astype(np.int64)
        wk = w[m]
        # rank of each edge within its (src, g) duplicate group
        r = _occurrence_ranks(sk_ * (G * MAXR) + gk)
        assert r.max(initial=0) < MAXR
        # row = (src, r): shared by all of src's rank-r edges (distinct g)
        row_key = sk_ * MAXR + r
        uniq, row_of_edge = np.unique(row_key, return_inverse=True)
        max_rows = max(max_rows, len(uniq))
        per_core.append((k, uniq, row_of_edge, gk, wk))

        n0, n1 = node_bounds[k], node_bounds[k + 1]
        bk = batch[n0:n1] - k * G
        pk = np.arange(n1 - n0) % P
        ranks = _occurrence_ranks(pk * G + bk)
        max_layers = max(max_layers, int(ranks.max(initial=-1)) + 1)

    tot_w = max(1, -(-max_rows // P))
    n_layers = max(1, max_layers)
    assert n_layers <= 6, n_layers

    in_maps = []
    for k, uniq, row_of_edge, gk, wk in per_core:
        nrows = len(uniq)
        row_src = uniq // MAXR  # the x row each window-row holds
        DG = D + G

        # packed per-window layout: [x block (96) | coeff block (64)]
        xc = np.zeros((P, tot_w * DG), dtype=lo)
        xr = np.zeros((tot_w * P, D), dtype=np.float32)
        xr[:nrows] = x[row_src]
        xr = xr.reshape(tot_w, P, D).transpose(1, 0, 2)  # [P, tot_w, D]
        xc.reshape(P, tot_w, DG)[:, :, :D] = xr.astype(lo)
        xc[row_of_edge % P, (row_of_edge // P) * DG + D + gk] = wk.astype(lo)

        # count layers: 0/1 placement, r-th occurrence of (p, batch) -> layer r
        n0, n1 = node_bounds[k], node_bounds[k + 1]
        bk = batch[n0:n1] - k * G
        pk = np.arange(n1 - n0) % P
        ranks = _occurrence_ranks(pk * G + bk)
        cm = np.zeros((P, n_layers * G), dtype=lo)
        cm[pk, ranks * G + bk] = 1.0

        in_maps.append(
            {
                "xc": xc,
                "cm": cm,
                "w1": np.asarray(W1, np.float32).reshape(D_FEAT, D_HID),
                "b1": np.asarray(b1, np.float32).reshape(D_HID, 1),
                "w2": np.asarray(W2, np.float32).reshape(D_HID, 1),
                "b2": np.asarray(b2, np.float32).reshape(1, 1),
            }
        )
    return in_maps, tot_w, n_layers


def get_nc(tot_w, n_layers, lo_name=None):
    lo_name = lo_name or LO_DT
    key = (tot_w, n_layers, lo_name)
    if key not in _nc_cache:
        _nc_cache[key] = _build_nc(tot_w, n_layers, lo_name)
    return _nc_cache[key]


def kernel(**inputs):
    from concourse import bass_utils

    in_maps, tot_w, n_layers = prepare_inputs(**inputs)
    nc = get_nc(tot_w, n_layers)
    res = bass_utils.run_bass_kernel_spmd(nc, in_maps, core_ids=list(range(CORES)))
    out = np.concatenate(
        [np.asarray(res.results[k]["out"], np.float32).reshape(GPC) for k in range(CORES)]
    )
    return out.reshape(N_GRAPHS, 1)
